# revision 15
# baseline (speedup 1.0000x reference)
"""Scatter-add of active-site feature rows into a dense (B, L, C) output,
distributed over 8 NeuronCores (data-parallel over the batch axis).

Core m owns flat output positions [m*8192, (m+1)*8192). Positions are
mapped to (group g, partition p, lane j) via  local = g*1024 + p*8 + j
(g<8, p<128, j<8), so a group's output tile [128 partitions, 8*512] stores
to DRAM with one contiguous 16 KB run per partition. On the host, rows are
bucketed by (core, g, j) "block" and padded to a uniform capacity Kc
(runtime max occupancy, ~104). On device each block's [128, 512] output
tile is a one-hot matmul

    out[p, c] = sum_k 1{lidx[k] == p} * feats[k, c]

which sums duplicate indices in fp32 PSUM and writes exact zeros for
untouched positions — every output element is produced by the kernel.
"""

import numpy as np

import concourse.bacc as bacc
import concourse.mybir as mybir
import concourse.tile as tile
from concourse.bass_utils import run_bass_kernel_spmd

N_CORES = 8
B = 16
L = 4096
C = 512
POS_PER_CORE = B * L // N_CORES  # 8192
import os
G_ENV = os.environ.get("K_G")  # force a specific G (testing only)
NBLK = 64  # blocks per core
# Buffer depths per G, sized to fit SBUF (ft/ot tiles are G*2KB/partition)
_BUFS = {2: (12, 8), 4: (10, 8), 8: (6, 4)}
FBUFS = int(os.environ.get("K_FBUFS", "0"))
OBUFS = int(os.environ.get("K_OBUFS", "0"))
CONST_RING = os.environ.get("K_CONST_RING", "sync")
MM_DTYPE = os.environ.get("K_MM_DTYPE", "float32")

_PROGRAM_CACHE: dict = {}


def _build_program(CH: int, Kc: int, G: int, FBUFS: int, OBUFS: int):
    NGRP = 64 // G
    f32 = mybir.dt.float32
    nc = bacc.Bacc(
        "TRN2",
        target_bir_lowering=False,
        debug=False,
        enable_asserts=False,
        num_devices=N_CORES,
    )
    split = MM_DTYPE in ("bf16split", "fp16split")
    fdt = {"bf16split": mybir.dt.bfloat16, "fp16split": mybir.dt.float16}.get(MM_DTYPE, f32)
    fwidth = 2 * C if split else C  # hi+lo halves per block when split
    feats_d = [
        nc.dram_tensor(f"feats{ch}", [Kc, NBLK * fwidth], fdt, kind="ExternalInput")
        for ch in range(CH)
    ]
    lidx_d = [
        nc.dram_tensor(f"lidx{ch}", [Kc, NBLK], f32, kind="ExternalInput")
        for ch in range(CH)
    ]
    iota_d = nc.dram_tensor("iota", [128, 128], f32, kind="ExternalInput")
    out_d = nc.dram_tensor("out", [POS_PER_CORE, C], f32, kind="ExternalOutput")

    eq = mybir.AluOpType.is_equal

    with tile.TileContext(nc) as tc:
        with (
            tc.tile_pool(name="const", bufs=1) as constp,
            tc.tile_pool(name="fpool", bufs=FBUFS) as fpool,
            tc.tile_pool(name="opool", bufs=OBUFS) as opool,
            tc.tile_pool(name="mpool", bufs=6) as mpool,
            tc.tile_pool(name="psum", bufs=8, space="PSUM") as pspool,
        ):
            const_eng = nc.sync if CONST_RING == "sync" else nc.scalar
            iota_t = constp.tile([128, 128], f32)
            const_eng.dma_start(iota_t[:], iota_d.ap())
            lidx_t = constp.tile([Kc, CH * NBLK], f32)
            for ch in range(CH):
                const_eng.dma_start(
                    lidx_t[:, ch * NBLK : (ch + 1) * NBLK], lidx_d[ch].ap()
                )

            # out viewed as [g, p, j, c]: row = g*1024 + p*8 + j
            out_v = out_d.ap().rearrange("(g p j) c -> g p (j c)", p=128, j=G)
            for g in range(NGRP):
                ftiles = []
                for ch in range(CH):
                    ft = fpool.tile([Kc, G * fwidth], fdt, tag="ft")
                    nc.sync.dma_start(
                        ft[:], feats_d[ch].ap()[:, g * G * fwidth : (g + 1) * G * fwidth]
                    )
                    ftiles.append(ft)
                ot = opool.tile([128, G * C], f32)
                for j in range(G):
                    b = g * G + j
                    ps = pspool.tile([128, C], f32)
                    for ch in range(CH):
                        m = mpool.tile([Kc, 128], fdt)
                        nc.vector.tensor_scalar(
                            m[:],
                            iota_t[:Kc, :],
                            lidx_t[:, ch * NBLK + b : ch * NBLK + b + 1],
                            None,
                            op0=eq,
                        )
                        if split:
                            base = j * 2 * C
                            nc.tensor.matmul(
                                ps[:], m[:], ftiles[ch][:, base : base + C],
                                start=(ch == 0), stop=False,
                            )
                            nc.tensor.matmul(
                                ps[:], m[:], ftiles[ch][:, base + C : base + 2 * C],
                                start=False, stop=(ch == CH - 1),
                            )
                        else:
                            lhsT = m[:]
                            rhs = ftiles[ch][:, j * C : (j + 1) * C]
                            if MM_DTYPE == "float32r":
                                lhsT = lhsT.bitcast(mybir.dt.float32r)
                                rhs = rhs.bitcast(mybir.dt.float32r)
                            nc.tensor.matmul(
                                ps[:],
                                lhsT,
                                rhs,
                                start=(ch == 0),
                                stop=(ch == CH - 1),
                            )
                    if j % 2 == 0:
                        nc.scalar.copy(ot[:, j * C : (j + 1) * C], ps[:])
                    else:
                        nc.vector.tensor_copy(ot[:, j * C : (j + 1) * C], ps[:])
                # store on the second HWDGE ring (ACT) to decouple from loads
                nc.scalar.dma_start(out_v[g], ot[:])

    nc.compile()
    return nc


def _block_decomposition(idx, G):
    core = idx >> 13  # // 8192
    local = idx & 8191
    g = local // (128 * G)  # position group
    rem = local % (128 * G)
    p = rem // G  # partition (position G-tuple)
    j = rem % G  # lane within tuple
    blk = g * G + j  # block id within core, 0..63
    gblk = core * NBLK + blk  # global block id, 0..511
    counts = np.bincount(gblk, minlength=N_CORES * NBLK)
    K = int(counts.max())
    CH = (K + 127) // 128
    Kc = -(-K // CH)  # ceil
    # Multiple of 32 keeps the HWDGE descriptor fan-out balanced across all
    # 16 SDMA engines (measured: Kc=92 concentrates loads on 4 engines and
    # costs +80 us; Kc=96 spreads them).
    Kc = (Kc + 31) & ~31
    return gblk, p, CH, Kc


def _prepare_inputs(input_features, site_indices):
    feats = np.ascontiguousarray(np.asarray(input_features, dtype=np.float32))
    idx = np.asarray(site_indices).astype(np.int64)
    n = idx.shape[0]
    assert feats.shape == (n, C)

    # The block composition (hence the padded capacity Kc) depends on the
    # lane count G; pick the G that minimizes transferred bytes for this
    # input, preferring larger DMA runs on ties.
    if G_ENV is not None:
        G = int(G_ENV)
        gblk, lpos, CH, Kc = _block_decomposition(idx, G)
    else:
        best = None
        for cand in (4, 2, 8):
            gblk_c, lpos_c, CH_c, Kc_c = _block_decomposition(idx, cand)
            if best is None or CH_c * Kc_c < best[0] * best[1]:
                best = (CH_c, Kc_c, cand, gblk_c, lpos_c)
        CH, Kc, G, gblk, lpos = best

    order = np.argsort(gblk, kind="stable")
    counts = np.bincount(gblk, minlength=N_CORES * NBLK)

    starts = np.zeros(N_CORES * NBLK, dtype=np.int64)
    np.cumsum(counts[:-1], out=starts[1:])
    slot = np.arange(n, dtype=np.int64) - np.repeat(starts, counts)

    g_sorted = gblk[order]
    core_s = g_sorted // NBLK
    blk_s = g_sorted % NBLK
    ch_s = slot // Kc
    k_s = slot - ch_s * Kc

    if MM_DTYPE in ("bf16split", "fp16split"):
        if MM_DTYPE == "bf16split":
            import ml_dtypes

            hdt = ml_dtypes.bfloat16
        else:
            hdt = np.float16
        feats_pack = np.zeros((N_CORES, CH, Kc, NBLK, 2, C), dtype=hdt)
        fs = feats[order]
        hi = fs.astype(hdt)
        lo = (fs - hi.astype(np.float32)).astype(hdt)
        feats_pack[core_s, ch_s, k_s, blk_s, 0, :] = hi
        feats_pack[core_s, ch_s, k_s, blk_s, 1, :] = lo
    else:
        feats_pack = np.zeros((N_CORES, CH, Kc, NBLK, C), dtype=np.float32)
        feats_pack[core_s, ch_s, k_s, blk_s, :] = feats[order]
    lidx_pack = np.full((N_CORES, CH, Kc, NBLK), -1.0, dtype=np.float32)
    lidx_pack[core_s, ch_s, k_s, blk_s] = lpos[order].astype(np.float32)

    iota = np.tile(np.arange(128, dtype=np.float32), (128, 1))

    in_maps = []
    for c in range(N_CORES):
        m = {"iota": iota}
        for ch in range(CH):
            m[f"feats{ch}"] = feats_pack[c, ch].reshape(Kc, -1)
            m[f"lidx{ch}"] = lidx_pack[c, ch]
        in_maps.append(m)
    return in_maps, CH, Kc, G


def run(input_features, site_indices, trace: bool = False):
    in_maps, CH, Kc, G = _prepare_inputs(input_features, site_indices)
    fbufs = FBUFS or _BUFS[G][0]
    obufs = OBUFS or _BUFS[G][1]
    key = (CH, Kc, G, fbufs, obufs, CONST_RING, MM_DTYPE)
    if key not in _PROGRAM_CACHE:
        _PROGRAM_CACHE[key] = _build_program(CH, Kc, G, fbufs, obufs)
    nc = _PROGRAM_CACHE[key]
    res = run_bass_kernel_spmd(nc, in_maps, list(range(N_CORES)), trace=trace)
    out = np.concatenate([res.results[c]["out"] for c in range(N_CORES)], axis=0)
    return out.reshape(B, L, C), res


def kernel(input_features, site_indices, batch_size, length):
    assert int(batch_size) == B and int(length) == L
    out, _ = run(input_features, site_indices, trace=False)
    return out


# revision 19
# speedup vs baseline: 1.0817x; 1.0817x over previous
"""Scatter-add of active-site feature rows into a dense (B, L, C) output,
distributed over 8 NeuronCores (data-parallel over the batch axis).

Core m owns flat output positions [m*8192, (m+1)*8192). Positions are
mapped to (group g, partition p, lane j) via  local = g*128*G + p*G + j
(p<128, j<G), so a group's output tile [128 partitions, G*512] stores to
DRAM with one contiguous G*2KB run per partition. On the host, rows are
bucketed by (core, g, j) "block" and padded to a uniform capacity Kc (the
runtime max block occupancy, rounded up to a multiple of 32 for DMA engine
fan-out); the lane count G is chosen per input to minimize Kc. On device
each block's [128, 512] output tile is a one-hot matmul

    out[p, c] = sum_k 1{lidx[k] == p} * feats[k, c]

which sums duplicate indices in fp32 PSUM and writes exact zeros for
untouched positions — every output element is produced by the kernel.
"""

import numpy as np

import concourse.bacc as bacc
import concourse.mybir as mybir
import concourse.tile as tile
from concourse.bass_utils import run_bass_kernel_spmd

N_CORES = 8
B = 16
L = 4096
C = 512
POS_PER_CORE = B * L // N_CORES  # 8192
import os
G_ENV = os.environ.get("K_G")  # force a specific G (testing only)
NBLK = 64  # blocks per core
# Buffer depths per G, sized to fit SBUF (ft/ot tiles are G*2KB/partition)
_BUFS = {2: (12, 8), 4: (10, 8), 8: (6, 4)}
FBUFS = int(os.environ.get("K_FBUFS", "0"))
OBUFS = int(os.environ.get("K_OBUFS", "0"))
CONST_RING = os.environ.get("K_CONST_RING", "sync")
MM_DTYPE = os.environ.get("K_MM_DTYPE", "float32")
COPY_ENG = os.environ.get("K_COPY", "dve")
STORE_MIX = int(os.environ.get("K_STORE_MIX", "0"))  # every Nth store on sync ring (0=off)

_PROGRAM_CACHE: dict = {}


def _build_program(CH: int, Kc: int, G: int, FBUFS: int, OBUFS: int):
    NGRP = 64 // G
    f32 = mybir.dt.float32
    nc = bacc.Bacc(
        "TRN2",
        target_bir_lowering=False,
        debug=False,
        enable_asserts=False,
        num_devices=N_CORES,
    )
    split = MM_DTYPE in ("bf16split", "fp16split")
    fdt = {"bf16split": mybir.dt.bfloat16, "fp16split": mybir.dt.float16}.get(MM_DTYPE, f32)
    fwidth = 2 * C if split else C  # hi+lo halves per block when split
    feats_d = [
        nc.dram_tensor(f"feats{ch}", [Kc, NBLK * fwidth], fdt, kind="ExternalInput")
        for ch in range(CH)
    ]
    lidx_d = [
        nc.dram_tensor(f"lidx{ch}", [Kc, NBLK], f32, kind="ExternalInput")
        for ch in range(CH)
    ]
    iota_d = nc.dram_tensor("iota", [128, 128], f32, kind="ExternalInput")
    out_d = nc.dram_tensor("out", [POS_PER_CORE, C], f32, kind="ExternalOutput")

    eq = mybir.AluOpType.is_equal

    with tile.TileContext(nc) as tc:
        with (
            tc.tile_pool(name="const", bufs=1) as constp,
            tc.tile_pool(name="fpool", bufs=FBUFS) as fpool,
            tc.tile_pool(name="opool", bufs=OBUFS) as opool,
            tc.tile_pool(name="mpool", bufs=6) as mpool,
            tc.tile_pool(name="psum", bufs=8, space="PSUM") as pspool,
        ):
            const_eng = nc.sync if CONST_RING == "sync" else nc.scalar
            iota_t = constp.tile([128, 128], f32)
            const_eng.dma_start(iota_t[:], iota_d.ap())
            lidx_t = constp.tile([Kc, CH * NBLK], f32)
            for ch in range(CH):
                const_eng.dma_start(
                    lidx_t[:, ch * NBLK : (ch + 1) * NBLK], lidx_d[ch].ap()
                )

            # out viewed as [g, p, j, c]: row = g*1024 + p*8 + j
            out_v = out_d.ap().rearrange("(g p j) c -> g p (j c)", p=128, j=G)
            for g in range(NGRP):
                ftiles = []
                for ch in range(CH):
                    ft = fpool.tile([Kc, G * fwidth], fdt, tag="ft")
                    nc.sync.dma_start(
                        ft[:], feats_d[ch].ap()[:, g * G * fwidth : (g + 1) * G * fwidth]
                    )
                    ftiles.append(ft)
                ot = opool.tile([128, G * C], f32)
                for j in range(G):
                    b = g * G + j
                    ps = pspool.tile([128, C], f32)
                    for ch in range(CH):
                        m = mpool.tile([Kc, 128], fdt)
                        nc.vector.tensor_scalar(
                            m[:],
                            iota_t[:Kc, :],
                            lidx_t[:, ch * NBLK + b : ch * NBLK + b + 1],
                            None,
                            op0=eq,
                        )
                        if split:
                            base = j * 2 * C
                            nc.tensor.matmul(
                                ps[:], m[:], ftiles[ch][:, base : base + C],
                                start=(ch == 0), stop=False,
                            )
                            nc.tensor.matmul(
                                ps[:], m[:], ftiles[ch][:, base + C : base + 2 * C],
                                start=False, stop=(ch == CH - 1),
                            )
                        else:
                            lhsT = m[:]
                            rhs = ftiles[ch][:, j * C : (j + 1) * C]
                            if MM_DTYPE == "float32r":
                                lhsT = lhsT.bitcast(mybir.dt.float32r)
                                rhs = rhs.bitcast(mybir.dt.float32r)
                            nc.tensor.matmul(
                                ps[:],
                                lhsT,
                                rhs,
                                start=(ch == 0),
                                stop=(ch == CH - 1),
                            )
                    if COPY_ENG == "mix" and j % 2 == 0:
                        nc.scalar.copy(ot[:, j * C : (j + 1) * C], ps[:])
                    else:
                        nc.vector.tensor_copy(ot[:, j * C : (j + 1) * C], ps[:])
                # store on the second HWDGE ring (ACT) to decouple from loads;
                # optionally rebalance a fraction onto the sync ring
                if STORE_MIX and g % STORE_MIX == STORE_MIX - 1:
                    nc.sync.dma_start(out_v[g], ot[:])
                else:
                    nc.scalar.dma_start(out_v[g], ot[:])

    nc.compile()
    return nc


def _block_decomposition(idx, G):
    core = idx >> 13  # // 8192
    local = idx & 8191
    g = local // (128 * G)  # position group
    rem = local % (128 * G)
    p = rem // G  # partition (position G-tuple)
    j = rem % G  # lane within tuple
    blk = g * G + j  # block id within core, 0..63
    gblk = core * NBLK + blk  # global block id, 0..511
    counts = np.bincount(gblk, minlength=N_CORES * NBLK)
    K = int(counts.max())
    CH = (K + 127) // 128
    Kc = -(-K // CH)  # ceil
    # Multiple of 32 keeps the HWDGE descriptor fan-out balanced across all
    # 16 SDMA engines (measured: Kc=92 concentrates loads on 4 engines and
    # costs +80 us; Kc=96 spreads them).
    Kc = (Kc + 31) & ~31
    return gblk, p, CH, Kc


def _prepare_inputs(input_features, site_indices):
    feats = np.ascontiguousarray(np.asarray(input_features, dtype=np.float32))
    idx = np.asarray(site_indices).astype(np.int64)
    n = idx.shape[0]
    assert feats.shape == (n, C)

    # The block composition (hence the padded capacity Kc) depends on the
    # lane count G; pick the G that minimizes transferred bytes for this
    # input, preferring larger DMA runs on ties.
    if G_ENV is not None:
        G = int(G_ENV)
        gblk, lpos, CH, Kc = _block_decomposition(idx, G)
    else:
        best = None
        for cand in (4, 2, 8):
            gblk_c, lpos_c, CH_c, Kc_c = _block_decomposition(idx, cand)
            if best is None or CH_c * Kc_c < best[0] * best[1]:
                best = (CH_c, Kc_c, cand, gblk_c, lpos_c)
        CH, Kc, G, gblk, lpos = best

    order = np.argsort(gblk, kind="stable")
    counts = np.bincount(gblk, minlength=N_CORES * NBLK)

    starts = np.zeros(N_CORES * NBLK, dtype=np.int64)
    np.cumsum(counts[:-1], out=starts[1:])
    slot = np.arange(n, dtype=np.int64) - np.repeat(starts, counts)

    g_sorted = gblk[order]
    core_s = g_sorted // NBLK
    blk_s = g_sorted % NBLK
    ch_s = slot // Kc
    k_s = slot - ch_s * Kc

    if MM_DTYPE in ("bf16split", "fp16split"):
        if MM_DTYPE == "bf16split":
            import ml_dtypes

            hdt = ml_dtypes.bfloat16
        else:
            hdt = np.float16
        feats_pack = np.zeros((N_CORES, CH, Kc, NBLK, 2, C), dtype=hdt)
        fs = feats[order]
        hi = fs.astype(hdt)
        lo = (fs - hi.astype(np.float32)).astype(hdt)
        feats_pack[core_s, ch_s, k_s, blk_s, 0, :] = hi
        feats_pack[core_s, ch_s, k_s, blk_s, 1, :] = lo
    else:
        feats_pack = np.zeros((N_CORES, CH, Kc, NBLK, C), dtype=np.float32)
        feats_pack[core_s, ch_s, k_s, blk_s, :] = feats[order]
    lidx_pack = np.full((N_CORES, CH, Kc, NBLK), -1.0, dtype=np.float32)
    lidx_pack[core_s, ch_s, k_s, blk_s] = lpos[order].astype(np.float32)

    iota = np.tile(np.arange(128, dtype=np.float32), (128, 1))

    in_maps = []
    for c in range(N_CORES):
        m = {"iota": iota}
        for ch in range(CH):
            m[f"feats{ch}"] = feats_pack[c, ch].reshape(Kc, -1)
            m[f"lidx{ch}"] = lidx_pack[c, ch]
        in_maps.append(m)
    return in_maps, CH, Kc, G


def run(input_features, site_indices, trace: bool = False):
    in_maps, CH, Kc, G = _prepare_inputs(input_features, site_indices)
    fbufs = FBUFS or _BUFS[G][0]
    obufs = OBUFS or _BUFS[G][1]
    key = (CH, Kc, G, fbufs, obufs, CONST_RING, MM_DTYPE, COPY_ENG, STORE_MIX)
    if key not in _PROGRAM_CACHE:
        _PROGRAM_CACHE[key] = _build_program(CH, Kc, G, fbufs, obufs)
    nc = _PROGRAM_CACHE[key]
    res = run_bass_kernel_spmd(nc, in_maps, list(range(N_CORES)), trace=trace)
    out = np.concatenate([res.results[c]["out"] for c in range(N_CORES)], axis=0)
    return out.reshape(B, L, C), res


def kernel(input_features, site_indices, batch_size, length):
    assert int(batch_size) == B and int(length) == L
    out, _ = run(input_features, site_indices, trace=False)
    return out
